# revision 34
# baseline (speedup 1.0000x reference)
"""Trainium2 Bass kernel for nn_BilinearSparseRouting (FC capsule routing layer).

Math (after constant-folding the softmax-over-a-constant, which is exactly 1/32):
    cp2[b,j]   = (pose[b,j] as 4x4) @ wc[j]            # (4,4) each
    S[b]       = (1/32) * sum_j cp2[b,j]               # (4,4)
    out[b,o]   = S[b] @ wn[o]                          # (4,4), o = 0..31
    output shape (256, 1, 1, 32, 16)

Device strategy (data-parallel over batch, 32 batches per core):
  Stage 1 is a 16384-term contraction per (b, r):
      T[(b,r), c] = sum_{(j,k)} pose[b, j, 4r+k] * wc[j, k, c]

  The end-to-end tolerance (2e-2) admits aggressive input quantization.
  pose is streamed as INT8 with a per-(b,r)-column scale (host-computed
  max/127): linear quantization of ~N(0,1) data gives ~1e-2 end-to-end
  error at 1 byte/element -- half the bytes of fp16, a quarter of fp32.
  The kernel is HBM-bound, so bytes are the objective: ~2.2 MiB/core.

  The PE cannot consume int8 directly, so each streamed group is cast
  int8 -> fp16 on-chip (integers up to +-127 are exact in fp16), with the
  work split across the otherwise-idle DVE / Activation / GPSIMD engines
  (~246 / 154 / ~90 elem/ns), keeping conversion ahead of both the DMA
  ring and the PE.  The per-column scale factors out of the whole
  contraction: stage 2's psum rows are (b,r), so one Activation copy with
  a per-partition scale vector applies it on the way out.

  PE structure: chunks of 128 contraction rows are PAIRED into one matmul,
      psum1[8, 256] += [wc_2p | wc_2p+1].T @ [xf_2p | xf_2p+1]
  so only the diagonal quadrants (0:4, 0:128) and (4:8, 128:256) carry the
  even/odd partial sums; the off-diagonal garbage is annihilated in stage
  2 by zero rows in the wn operand.  64 matmuls at the baseline-proven
  256-column cadence; a short warm-up chain on zeroed SBUF ramps the PE
  p-state (1.2 -> 2.4 GHz) before real data lands.

  Stage 2 downcasts psum1 to a [8, 256] fp16 tile and contracts against
  wn/32 (host-prescaled, exact power of 2) in two small fp16 matmuls
  accumulating into one [128, 128] psum.

  The x stream is laid out on the host as per-group dense contiguous DRAM
  regions, all on the sync HWDGE ring (a second ring adds no bandwidth --
  the 16 DMA engines are shared -- and out-of-order delivery stalls the
  PE); the scale vector and a weight header ride separate early DMAs.
"""

import os
import sys

for _p in ("/opt/trn_rl_repo", "/root/.axon_site/_ro/trn_rl_repo"):
    if _p not in sys.path:
        sys.path.insert(0, _p)

# The kernel executes through the axon PJRT backend; a leftover cpu pin from a
# reference-running harness would hide the NeuronCores if jax has not
# initialized its backend yet.
os.environ.pop("JAX_PLATFORMS", None)

from contextlib import ExitStack  # noqa: E402

import numpy as np  # noqa: E402

import concourse.bacc as bacc  # noqa: E402
import concourse.mybir as mybir  # noqa: E402
import concourse.tile as tile  # noqa: E402
from concourse.bass_utils import run_bass_kernel_spmd  # noqa: E402

B = 256
N_IN = 4096
N_OUT = 32
MPD = 4
POSE_DIM = 16
N_CORES = 8
B_SH = B // N_CORES            # 32 batches per core
JK = N_IN * MPD                # 16384 contraction terms
NCHUNK = JK // 128             # 128 contraction chunks of 128 rows
NQUAD = NCHUNK // 4            # 32 quad matmuls
XCOLS = NCHUNK * 128           # packed int8 columns of x
W4 = NCHUNK * 4                # stage-1 weight columns (4 per chunk)
WNC = 512                      # wn block columns in header (4 phase blocks)

F32 = mybir.dt.float32
F16 = mybir.dt.float16
I8 = mybir.dt.int8

# Built once, reused across kernel() calls.
_CACHE = {}

# test.py hooks: set TRACE=True before calling kernel() to profile; the
# BassKernelResults of the last run lands in LAST_RESULT.
TRACE = False
TRACE_KWARGS = {}
LAST_RESULT = None

# x group boundaries in chunks (all deltas divisible by 4 so quad matmuls
# never span a group).  At most 7 groups: the software DGE tracks in-flight
# direct DMAs and an 8th gpsimd dma_start triggers a multi-us drain of the
# ring.  Small first group so stage 1 starts early, smaller last group so
# the PE trail after the last byte lands is short.
BOUNDS = [0, 12, 36, 56, 76, 96, 116, 128]

# Dummy 256-column matmuls on zeroed SBUF, run while the stream's first
# groups are still in flight: the PE p-state ramps with busy time, and a
# cold PE runs stage 1 at half the column rate for several microseconds.
# Sized so the chain ends just AFTER the first x group lands -- an idle
# gap between warm-up and stage 1 resets the p-state ramp.
N_WARM = 17


def _cvt_split(npairs):
    """Split a group's pair range across DVE / Activation / GPSIMD in
    rough proportion to their elementwise copy rates."""
    nd = (npairs + 1) // 2
    na = max((npairs - nd) * 2 // 3, 0)
    return nd, na, npairs - nd - na


def _build_program():
    nc = bacc.Bacc("TRN2", target_bir_lowering=False, debug=False,
                   num_devices=N_CORES)
    y = nc.dram_tensor("y", [128, 128], F32, kind="ExternalOutput").ap()

    bounds = BOUNDS
    assert bounds[-1] == NCHUNK

    hdr_t = nc.dram_tensor("hdr", [128, W4 + WNC], F16,
                           kind="ExternalInput").ap()
    sv_t = nc.dram_tensor("sv", [128, 1], F32, kind="ExternalInput").ap()
    xg = [
        nc.dram_tensor(
            f"x{g + 1}",
            [128, (bounds[g + 1] - bounds[g]) * 128],
            I8, kind="ExternalInput").ap()
        for g in range(len(bounds) - 1)
    ]

    with tile.TileContext(nc) as tc, ExitStack() as ctx:
        xpool = ctx.enter_context(tc.tile_pool(name="xpool", bufs=1))
        opool = ctx.enter_context(tc.tile_pool(name="opool", bufs=1))
        ppool = ctx.enter_context(tc.tile_pool(name="ppool", bufs=1, space="PSUM"))

        # Header (stage-1/2 weights) and scale vector ride ahead of the
        # int8 stream: header first on the sync ring, scales on the scalar
        # ring (otherwise idle).
        hdr_sb = xpool.tile([128, W4 + WNC], F16, tag="hdr")
        nc.sync.dma_start(hdr_sb[:], hdr_t[:])
        sv_sb = opool.tile([128, 1], F32, tag="sv")
        nc.scalar.dma_start(sv_sb[:], sv_t[:])

        # PE warm-up: emitted before the stream doorbells so the scheduler
        # places the gpsimd memset at the head of that engine's queue; the
        # chain then issues right after the engine prologues, several us
        # before the first x group is ready.  The zero products stay in a
        # scratch psum that is never read.
        warm = opool.tile([128, 256], F16, tag="warm")
        nc.gpsimd.memset(warm[:], 0)
        psum_w = ppool.tile([8, 256], F32, tag="warmp")
        for i in range(N_WARM):
            nc.tensor.matmul(psum_w[:], lhsT=warm[:, 0:8], rhs=warm[:],
                             start=(i == 0), stop=(i == N_WARM - 1))

        n_groups = len(bounds) - 1
        xfs = []
        for g in range(n_groups):
            ncols = (bounds[g + 1] - bounds[g]) * 128
            # Casting DMA (software DGE): the DMA path upconverts int8 ->
            # fp16 in flight, so HBM sees 1 byte/element and no compute
            # engine touches the data before the PE.
            xf = xpool.tile([128, ncols], F16, tag=f"xf{g}")
            nc.gpsimd.dma_start(xf[:], xg[g][:])
            xfs.append(xf)
        w_sb = hdr_sb[:, 0:W4]
        wn_sb = hdr_sb[0:16, W4:W4 + WNC]

        # Stage 1: 32 quad 512-column fp16 matmuls, alternating between two
        # psum banks so two can be in flight (back-to-back accumulation
        # into a single psum serializes at duration; alternating hides the
        # per-instruction drain latency and beats the Tensor sequencer's
        # ~131 ns LDWEIGHTS+MATMUL floor per 256 columns).  Chunk 4q+a
        # accumulates its partial S into diagonal block (4a:4a+4,
        # 128a:128a+128); off-diagonal blocks are garbage, neutralized in
        # stage 2 by zero rows in wn.
        psum1a = ppool.tile([16, 512], F32, tag="ta")
        psum1b = ppool.tile([16, 512], F32, tag="tb")
        psum1 = [psum1a, psum1b]
        for g in range(n_groups):
            c0, c1 = bounds[g], bounds[g + 1]
            xf = xfs[g]
            for qq in range((c1 - c0) // 4):
                q = c0 // 4 + qq
                nc.tensor.matmul(
                    psum1[q % 2][:],
                    lhsT=w_sb[:, q * 16:(q + 1) * 16],
                    rhs=xf[:, qq * 512:(qq + 1) * 512],
                    start=(q < 2),
                    stop=(q >= NQUAD - 2),
                )

        # Downcast both psums (live + garbage blocks) to fp16 in
        # partition-0-aligned copies; compute engines cannot start an
        # access at partition 4, so the garbage blocks are neutralized in
        # stage 2 by zero rows in the wn operand.
        s16 = opool.tile([16, 1024], F16, tag="s16")
        nc.vector.tensor_copy(s16[:, 0:512], psum1[0][:])
        nc.vector.tensor_copy(s16[:, 512:1024], psum1[1][:])

        # Stage 2: contract over (k2, phase) against wn/32; block a of the
        # wn header has wn in rows 4a:4a+4 and zeros elsewhere.
        psum2 = ppool.tile([128, 128], F32, tag="out")
        for h in range(8):
            a = h % 4
            nc.tensor.matmul(psum2[:],
                             lhsT=s16[:, h * 128:(h + 1) * 128],
                             rhs=wn_sb[:, a * 128:(a + 1) * 128],
                             start=(h == 0), stop=(h == 7))
        # Apply the per-(b,r) dequant scale on the way out: psum2 rows are
        # (b,r), so a per-partition scale vector does it in one copy.
        out_sb = opool.tile([128, 128], F32, tag="y")
        nc.scalar.mul(out_sb[:], psum2[:], sv_sb[:])
        nc.sync.dma_start(y[:], out_sb[:])

    nc.compile()
    return nc


def _prep_x(current_pose: np.ndarray):
    """(256, 4096, 16) -> per-core int8 chunk images + fp32 column scales.

    Per core the stage-1 contraction matrix has row index (j*4 + k) and
    column (b*4 + r) with element pose[b, j, 4r+k].  Chunk Jc's 128x128
    tile lands in packed columns [Jc*128, (Jc+1)*128).
    """
    a = current_pose.reshape(N_CORES, B_SH, N_IN, MPD, MPD)   # m b j r k
    t = a.transpose(0, 2, 4, 1, 3)                            # m j k b r
    X = t.reshape(N_CORES, JK, 128)                           # m (jk) (b,r)
    s = (np.abs(X).max(axis=1) / np.float32(127.0)).astype(np.float32)
    q = np.clip(np.rint(X / s[:, None, :]), -127, 127).astype(np.int8)
    c = q.reshape(N_CORES, NCHUNK, 128, 128)                  # m Jc p col
    xs = np.ascontiguousarray(
        c.transpose(0, 2, 1, 3).reshape(N_CORES, 128, XCOLS))
    return xs, s


def kernel(current_pose, w_current, w_next, h_out=1, w_out=1):
    global LAST_RESULT
    current_pose = np.asarray(current_pose, dtype=np.float32)
    w_current = np.asarray(w_current, dtype=np.float32)
    w_next = np.asarray(w_next, dtype=np.float32)

    if not TRACE:
        # bass_utils would honor a stray BASS_TRACE env var and then crash on
        # this image's missing NTFF hook module.
        os.environ.pop("BASS_TRACE", None)

    if "nc" not in _CACHE:
        _CACHE["nc"] = _build_program()
    nc = _CACHE["nc"]
    bounds = BOUNDS

    xs, s = _prep_x(current_pose)

    # wc[j,k,c] flattened over rows (j,k); chunk Jc's (128, 4) block packed
    # into header columns [Jc*4, (Jc+1)*4).
    wc_flat = w_current.reshape(JK, MPD).astype(np.float16)
    w_img = np.ascontiguousarray(
        wc_flat.reshape(NCHUNK, 128, MPD).transpose(1, 0, 2).reshape(128, W4))

    # wn arranged (k2, (o,c)), pre-scaled by the exact 1/32 softmax
    # constant, in four phase blocks: block a carries wn in rows 4a:4a+4;
    # the complementary rows stay zero to kill the psum1 garbage blocks in
    # stage 2.
    wn4 = (w_next.transpose(1, 0, 2).reshape(MPD, N_OUT * MPD)
           * np.float32(1.0 / N_OUT)).astype(np.float16)
    wn_img = np.zeros((128, WNC), dtype=np.float16)
    for a in range(4):
        wn_img[a * MPD:(a + 1) * MPD, a * 128:(a + 1) * 128] = wn4
    hdr = np.ascontiguousarray(np.concatenate([w_img, wn_img], axis=1))

    in_maps = [
        {"hdr": hdr,
         "sv": np.ascontiguousarray(s[m][:, None]),
         **{f"x{g + 1}": np.ascontiguousarray(
                xs[m][:, bounds[g] * 128:bounds[g + 1] * 128])
            for g in range(len(bounds) - 1)}}
        for m in range(N_CORES)
    ]
    res = run_bass_kernel_spmd(nc, in_maps, list(range(N_CORES)), trace=TRACE,
                               **TRACE_KWARGS)
    LAST_RESULT = res

    out = np.empty((B, 1, 1, N_OUT, POSE_DIM), dtype=np.float32)
    for m in range(N_CORES):
        ym = res.results[m]["y"]                      # (128=(b,r), 128=(o,c))
        out[m * B_SH:(m + 1) * B_SH, 0, 0] = (
            ym.reshape(B_SH, MPD, N_OUT, MPD)
            .transpose(0, 2, 1, 3).reshape(B_SH, N_OUT, POSE_DIM))
    return out


# revision 36
# speedup vs baseline: 1.0340x; 1.0340x over previous
"""Trainium2 Bass kernel for nn_BilinearSparseRouting (FC capsule routing layer).

Math (after constant-folding the softmax-over-a-constant, which is exactly 1/32):
    cp2[b,j]   = (pose[b,j] as 4x4) @ wc[j]            # (4,4) each
    S[b]       = (1/32) * sum_j cp2[b,j]               # (4,4)
    out[b,o]   = S[b] @ wn[o]                          # (4,4), o = 0..31
    output shape (256, 1, 1, 32, 16)

Device strategy (data-parallel over batch, 32 batches per core):
  Stage 1 is a 16384-term contraction per (b, r):
      T[(b,r), c] = sum_{(j,k)} pose[b, j, 4r+k] * wc[j, k, c]

  The end-to-end tolerance (2e-2) admits aggressive input quantization.
  pose is streamed as INT8 with a per-(b,r)-column scale (host-computed
  max/127): linear quantization of ~N(0,1) data gives ~1e-2 end-to-end
  error at 1 byte/element -- half the bytes of fp16, a quarter of fp32.
  The kernel is HBM-bound, so bytes are the objective: ~2.2 MiB/core.

  The PE cannot consume int8 directly, so each streamed group is cast
  int8 -> fp16 on-chip (integers up to +-127 are exact in fp16), with the
  work split across the otherwise-idle DVE / Activation / GPSIMD engines
  (~246 / 154 / ~90 elem/ns), keeping conversion ahead of both the DMA
  ring and the PE.  The per-column scale factors out of the whole
  contraction: stage 2's psum rows are (b,r), so one Activation copy with
  a per-partition scale vector applies it on the way out.

  PE structure: chunks of 128 contraction rows are PAIRED into one matmul,
      psum1[8, 256] += [wc_2p | wc_2p+1].T @ [xf_2p | xf_2p+1]
  so only the diagonal quadrants (0:4, 0:128) and (4:8, 128:256) carry the
  even/odd partial sums; the off-diagonal garbage is annihilated in stage
  2 by zero rows in the wn operand.  64 matmuls at the baseline-proven
  256-column cadence; a short warm-up chain on zeroed SBUF ramps the PE
  p-state (1.2 -> 2.4 GHz) before real data lands.

  Stage 2 downcasts psum1 to a [8, 256] fp16 tile and contracts against
  wn/32 (host-prescaled, exact power of 2) in two small fp16 matmuls
  accumulating into one [128, 128] psum.

  The x stream is laid out on the host as per-group dense contiguous DRAM
  regions, all on the sync HWDGE ring (a second ring adds no bandwidth --
  the 16 DMA engines are shared -- and out-of-order delivery stalls the
  PE); the scale vector and a weight header ride separate early DMAs.
"""

import os
import sys

for _p in ("/opt/trn_rl_repo", "/root/.axon_site/_ro/trn_rl_repo"):
    if _p not in sys.path:
        sys.path.insert(0, _p)

# The kernel executes through the axon PJRT backend; a leftover cpu pin from a
# reference-running harness would hide the NeuronCores if jax has not
# initialized its backend yet.
os.environ.pop("JAX_PLATFORMS", None)

from contextlib import ExitStack  # noqa: E402

import numpy as np  # noqa: E402

import concourse.bacc as bacc  # noqa: E402
import concourse.mybir as mybir  # noqa: E402
import concourse.tile as tile  # noqa: E402
from concourse.bass_utils import run_bass_kernel_spmd  # noqa: E402

B = 256
N_IN = 4096
N_OUT = 32
MPD = 4
POSE_DIM = 16
N_CORES = 8
B_SH = B // N_CORES            # 32 batches per core
JK = N_IN * MPD                # 16384 contraction terms
NCHUNK = JK // 128             # 128 contraction chunks of 128 rows
NQUAD = NCHUNK // 4            # 32 quad matmuls
XCOLS = NCHUNK * 128           # packed int8 columns of x
W4 = NCHUNK * 4                # stage-1 weight columns (4 per chunk)
WNC = 512                      # wn block columns in header (4 phase blocks)

F32 = mybir.dt.float32
F16 = mybir.dt.float16
I8 = mybir.dt.int8

# Built once, reused across kernel() calls.
_CACHE = {}

# test.py hooks: set TRACE=True before calling kernel() to profile; the
# BassKernelResults of the last run lands in LAST_RESULT.
TRACE = False
TRACE_KWARGS = {}
LAST_RESULT = None

# x group boundaries in chunks (all deltas divisible by 4 so quad matmuls
# never span a group).  At most 7 groups: the software DGE tracks in-flight
# direct DMAs and an 8th gpsimd dma_start triggers a multi-us drain of the
# ring.  Small first group so stage 1 starts early, smaller last group so
# the PE trail after the last byte lands is short.
BOUNDS = [0, 12, 36, 60, 84, 104, 120, 128]

# Dummy 256-column matmuls on zeroed SBUF, run while the stream's first
# groups are still in flight: the PE p-state ramps with busy time, and a
# cold PE runs stage 1 at half the column rate for several microseconds.
# Sized so the chain ends just AFTER the first x group lands -- an idle
# gap between warm-up and stage 1 resets the p-state ramp.
N_WARM = 17


def _cvt_split(npairs):
    """Split a group's pair range across DVE / Activation / GPSIMD in
    rough proportion to their elementwise copy rates."""
    nd = (npairs + 1) // 2
    na = max((npairs - nd) * 2 // 3, 0)
    return nd, na, npairs - nd - na


def _build_program():
    nc = bacc.Bacc("TRN2", target_bir_lowering=False, debug=False,
                   num_devices=N_CORES)
    y = nc.dram_tensor("y", [128, 128], F32, kind="ExternalOutput").ap()

    bounds = BOUNDS
    assert bounds[-1] == NCHUNK

    hdr_t = nc.dram_tensor("hdr", [128, W4 + WNC], F16,
                           kind="ExternalInput").ap()
    sv_t = nc.dram_tensor("sv", [128, 1], F32, kind="ExternalInput").ap()
    xg = [
        nc.dram_tensor(
            f"x{g + 1}",
            [128, (bounds[g + 1] - bounds[g]) * 128],
            I8, kind="ExternalInput").ap()
        for g in range(len(bounds) - 1)
    ]

    with tile.TileContext(nc) as tc, ExitStack() as ctx:
        xpool = ctx.enter_context(tc.tile_pool(name="xpool", bufs=1))
        opool = ctx.enter_context(tc.tile_pool(name="opool", bufs=1))
        ppool = ctx.enter_context(tc.tile_pool(name="ppool", bufs=1, space="PSUM"))

        # Header (stage-1/2 weights) and scale vector ride ahead of the
        # int8 stream: header first on the sync ring, scales on the scalar
        # ring (otherwise idle).
        hdr_sb = xpool.tile([128, W4 + WNC], F16, tag="hdr")
        nc.sync.dma_start(hdr_sb[:], hdr_t[:])
        sv_sb = opool.tile([128, 1], F32, tag="sv")
        nc.scalar.dma_start(sv_sb[:], sv_t[:])

        # PE warm-up: emitted before the stream doorbells so the scheduler
        # places the gpsimd memset at the head of that engine's queue; the
        # chain then issues right after the engine prologues, several us
        # before the first x group is ready.  The zero products stay in a
        # scratch psum that is never read.
        warm = opool.tile([128, 256], F16, tag="warm")
        nc.gpsimd.memset(warm[:], 0)
        psum_w = ppool.tile([8, 256], F32, tag="warmp")
        for i in range(N_WARM):
            nc.tensor.matmul(psum_w[:], lhsT=warm[:, 0:8], rhs=warm[:],
                             start=(i == 0), stop=(i == N_WARM - 1))

        n_groups = len(bounds) - 1
        xfs = []
        for g in range(n_groups):
            ncols = (bounds[g + 1] - bounds[g]) * 128
            # Casting DMA (software DGE): the DMA path upconverts int8 ->
            # fp16 in flight, so HBM sees 1 byte/element and no compute
            # engine touches the data before the PE.
            xf = xpool.tile([128, ncols], F16, tag=f"xf{g}")
            nc.gpsimd.dma_start(xf[:], xg[g][:])
            xfs.append(xf)
        w_sb = hdr_sb[:, 0:W4]
        wn_sb = hdr_sb[0:16, W4:W4 + WNC]

        # Stage 1: 32 quad 512-column fp16 matmuls, alternating between two
        # psum banks so two can be in flight (back-to-back accumulation
        # into a single psum serializes at duration; alternating hides the
        # per-instruction drain latency and beats the Tensor sequencer's
        # ~131 ns LDWEIGHTS+MATMUL floor per 256 columns).  Chunk 4q+a
        # accumulates its partial S into diagonal block (4a:4a+4,
        # 128a:128a+128); off-diagonal blocks are garbage, neutralized in
        # stage 2 by zero rows in wn.
        psum1a = ppool.tile([16, 512], F32, tag="ta")
        psum1b = ppool.tile([16, 512], F32, tag="tb")
        psum1 = [psum1a, psum1b]
        for g in range(n_groups):
            c0, c1 = bounds[g], bounds[g + 1]
            xf = xfs[g]
            for qq in range((c1 - c0) // 4):
                q = c0 // 4 + qq
                nc.tensor.matmul(
                    psum1[q % 2][:],
                    lhsT=w_sb[:, q * 16:(q + 1) * 16],
                    rhs=xf[:, qq * 512:(qq + 1) * 512],
                    start=(q < 2),
                    stop=(q >= NQUAD - 2),
                )

        # Downcast both psums (live + garbage blocks) to fp16 in
        # partition-0-aligned copies; compute engines cannot start an
        # access at partition 4, so the garbage blocks are neutralized in
        # stage 2 by zero rows in the wn operand.
        s16 = opool.tile([16, 1024], F16, tag="s16")
        nc.vector.tensor_copy(s16[:, 0:512], psum1[0][:])
        nc.scalar.copy(s16[:, 512:1024], psum1[1][:])

        # Stage 2: contract over (k2, phase) against wn/32; block a of the
        # wn header has wn in rows 4a:4a+4 and zeros elsewhere.
        psum2 = ppool.tile([128, 128], F32, tag="out")
        for h in range(8):
            a = h % 4
            nc.tensor.matmul(psum2[:],
                             lhsT=s16[:, h * 128:(h + 1) * 128],
                             rhs=wn_sb[:, a * 128:(a + 1) * 128],
                             start=(h == 0), stop=(h == 7))
        # Apply the per-(b,r) dequant scale on the way out: psum2 rows are
        # (b,r), so a per-partition scale vector does it in one copy.
        out_sb = opool.tile([128, 128], F32, tag="y")
        nc.scalar.mul(out_sb[:], psum2[:], sv_sb[:])
        nc.sync.dma_start(y[:], out_sb[:])

    nc.compile()
    return nc


def _prep_x(current_pose: np.ndarray):
    """(256, 4096, 16) -> per-core int8 chunk images + fp32 column scales.

    Per core the stage-1 contraction matrix has row index (j*4 + k) and
    column (b*4 + r) with element pose[b, j, 4r+k].  Chunk Jc's 128x128
    tile lands in packed columns [Jc*128, (Jc+1)*128).
    """
    a = current_pose.reshape(N_CORES, B_SH, N_IN, MPD, MPD)   # m b j r k
    t = a.transpose(0, 2, 4, 1, 3)                            # m j k b r
    X = t.reshape(N_CORES, JK, 128)                           # m (jk) (b,r)
    s = (np.abs(X).max(axis=1) / np.float32(127.0)).astype(np.float32)
    q = np.clip(np.rint(X / s[:, None, :]), -127, 127).astype(np.int8)
    c = q.reshape(N_CORES, NCHUNK, 128, 128)                  # m Jc p col
    xs = np.ascontiguousarray(
        c.transpose(0, 2, 1, 3).reshape(N_CORES, 128, XCOLS))
    return xs, s


def kernel(current_pose, w_current, w_next, h_out=1, w_out=1):
    global LAST_RESULT
    current_pose = np.asarray(current_pose, dtype=np.float32)
    w_current = np.asarray(w_current, dtype=np.float32)
    w_next = np.asarray(w_next, dtype=np.float32)

    if not TRACE:
        # bass_utils would honor a stray BASS_TRACE env var and then crash on
        # this image's missing NTFF hook module.
        os.environ.pop("BASS_TRACE", None)

    if "nc" not in _CACHE:
        _CACHE["nc"] = _build_program()
    nc = _CACHE["nc"]
    bounds = BOUNDS

    xs, s = _prep_x(current_pose)

    # wc[j,k,c] flattened over rows (j,k); chunk Jc's (128, 4) block packed
    # into header columns [Jc*4, (Jc+1)*4).
    wc_flat = w_current.reshape(JK, MPD).astype(np.float16)
    w_img = np.ascontiguousarray(
        wc_flat.reshape(NCHUNK, 128, MPD).transpose(1, 0, 2).reshape(128, W4))

    # wn arranged (k2, (o,c)), pre-scaled by the exact 1/32 softmax
    # constant, in four phase blocks: block a carries wn in rows 4a:4a+4;
    # the complementary rows stay zero to kill the psum1 garbage blocks in
    # stage 2.
    wn4 = (w_next.transpose(1, 0, 2).reshape(MPD, N_OUT * MPD)
           * np.float32(1.0 / N_OUT)).astype(np.float16)
    wn_img = np.zeros((128, WNC), dtype=np.float16)
    for a in range(4):
        wn_img[a * MPD:(a + 1) * MPD, a * 128:(a + 1) * 128] = wn4
    hdr = np.ascontiguousarray(np.concatenate([w_img, wn_img], axis=1))

    in_maps = [
        {"hdr": hdr,
         "sv": np.ascontiguousarray(s[m][:, None]),
         **{f"x{g + 1}": np.ascontiguousarray(
                xs[m][:, bounds[g] * 128:bounds[g + 1] * 128])
            for g in range(len(bounds) - 1)}}
        for m in range(N_CORES)
    ]
    res = run_bass_kernel_spmd(nc, in_maps, list(range(N_CORES)), trace=TRACE,
                               **TRACE_KWARGS)
    LAST_RESULT = res

    out = np.empty((B, 1, 1, N_OUT, POSE_DIM), dtype=np.float32)
    for m in range(N_CORES):
        ym = res.results[m]["y"]                      # (128=(b,r), 128=(o,c))
        out[m * B_SH:(m + 1) * B_SH, 0, 0] = (
            ym.reshape(B_SH, MPD, N_OUT, MPD)
            .transpose(0, 2, 1, 3).reshape(B_SH, N_OUT, POSE_DIM))
    return out


# revision 37
# speedup vs baseline: 1.0420x; 1.0077x over previous
"""Trainium2 Bass kernel for nn_BilinearSparseRouting (FC capsule routing layer).

Math (after constant-folding the softmax-over-a-constant, which is exactly 1/32):
    cp2[b,j]   = (pose[b,j] as 4x4) @ wc[j]            # (4,4) each
    S[b]       = (1/32) * sum_j cp2[b,j]               # (4,4)
    out[b,o]   = S[b] @ wn[o]                          # (4,4), o = 0..31
    output shape (256, 1, 1, 32, 16)

Device strategy (data-parallel over batch, 32 batches per core):
  Stage 1 is a 16384-term contraction per (b, r):
      T[(b,r), c] = sum_{(j,k)} pose[b, j, 4r+k] * wc[j, k, c]

  The end-to-end tolerance (2e-2) admits aggressive input quantization.
  pose is streamed as INT8 with a per-(b,r)-column scale (host-computed
  max/127): linear quantization of ~N(0,1) data gives ~1e-2 end-to-end
  error at 1 byte/element -- half the bytes of fp16, a quarter of fp32.
  The kernel is HBM-bound, so bytes are the objective: ~2.2 MiB/core.

  The PE cannot consume int8 directly, so each streamed group is cast
  int8 -> fp16 on-chip (integers up to +-127 are exact in fp16), with the
  work split across the otherwise-idle DVE / Activation / GPSIMD engines
  (~246 / 154 / ~90 elem/ns), keeping conversion ahead of both the DMA
  ring and the PE.  The per-column scale factors out of the whole
  contraction: stage 2's psum rows are (b,r), so one Activation copy with
  a per-partition scale vector applies it on the way out.

  PE structure: chunks of 128 contraction rows are PAIRED into one matmul,
      psum1[8, 256] += [wc_2p | wc_2p+1].T @ [xf_2p | xf_2p+1]
  so only the diagonal quadrants (0:4, 0:128) and (4:8, 128:256) carry the
  even/odd partial sums; the off-diagonal garbage is annihilated in stage
  2 by zero rows in the wn operand.  64 matmuls at the baseline-proven
  256-column cadence; a short warm-up chain on zeroed SBUF ramps the PE
  p-state (1.2 -> 2.4 GHz) before real data lands.

  Stage 2 downcasts psum1 to a [8, 256] fp16 tile and contracts against
  wn/32 (host-prescaled, exact power of 2) in two small fp16 matmuls
  accumulating into one [128, 128] psum.

  The x stream is laid out on the host as per-group dense contiguous DRAM
  regions, all on the sync HWDGE ring (a second ring adds no bandwidth --
  the 16 DMA engines are shared -- and out-of-order delivery stalls the
  PE); the scale vector and a weight header ride separate early DMAs.
"""

import os
import sys

for _p in ("/opt/trn_rl_repo", "/root/.axon_site/_ro/trn_rl_repo"):
    if _p not in sys.path:
        sys.path.insert(0, _p)

# The kernel executes through the axon PJRT backend; a leftover cpu pin from a
# reference-running harness would hide the NeuronCores if jax has not
# initialized its backend yet.
os.environ.pop("JAX_PLATFORMS", None)

from contextlib import ExitStack  # noqa: E402

import numpy as np  # noqa: E402

import concourse.bacc as bacc  # noqa: E402
import concourse.mybir as mybir  # noqa: E402
import concourse.tile as tile  # noqa: E402
from concourse.bass_utils import run_bass_kernel_spmd  # noqa: E402

B = 256
N_IN = 4096
N_OUT = 32
MPD = 4
POSE_DIM = 16
N_CORES = 8
B_SH = B // N_CORES            # 32 batches per core
JK = N_IN * MPD                # 16384 contraction terms
NCHUNK = JK // 128             # 128 contraction chunks of 128 rows
NPAIR = NCHUNK // 2            # 64 pair matmuls
XCOLS = NCHUNK * 128           # packed int8 columns of x
W4 = NCHUNK * 4                # stage-1 weight columns (4 per chunk)
WNC = 256                      # wn block columns in header (2 parity blocks)

F32 = mybir.dt.float32
F16 = mybir.dt.float16
I8 = mybir.dt.int8

# Built once, reused across kernel() calls.
_CACHE = {}

# test.py hooks: set TRACE=True before calling kernel() to profile; the
# BassKernelResults of the last run lands in LAST_RESULT.
TRACE = False
TRACE_KWARGS = {}
LAST_RESULT = None

# x group boundaries in chunks (all deltas even so pair matmuls never span
# a group).  At most 7 groups: the software DGE tracks in-flight direct
# DMAs and an 8th gpsimd dma_start triggers a multi-us drain of the ring.
# Small first group so stage 1 starts early, smaller last group so the PE
# trail after the last byte lands is short.
BOUNDS = [0, 8, 30, 52, 74, 96, 114, 128]

# Dummy 256-column matmuls on zeroed SBUF, run while the stream's first
# groups are still in flight: the PE p-state ramps with busy time, and a
# cold PE runs stage 1 at half the column rate for several microseconds.
# Sized so the chain ends just AFTER the first x group lands -- an idle
# gap between warm-up and stage 1 resets the p-state ramp.
N_WARM = 17


def _cvt_split(npairs):
    """Split a group's pair range across DVE / Activation / GPSIMD in
    rough proportion to their elementwise copy rates."""
    nd = (npairs + 1) // 2
    na = max((npairs - nd) * 2 // 3, 0)
    return nd, na, npairs - nd - na


def _build_program():
    nc = bacc.Bacc("TRN2", target_bir_lowering=False, debug=False,
                   num_devices=N_CORES)
    y = nc.dram_tensor("y", [128, 128], F32, kind="ExternalOutput").ap()

    bounds = BOUNDS
    assert bounds[-1] == NCHUNK

    hdr_t = nc.dram_tensor("hdr", [128, W4 + WNC], F16,
                           kind="ExternalInput").ap()
    sv_t = nc.dram_tensor("sv", [128, 1], F32, kind="ExternalInput").ap()
    xg = [
        nc.dram_tensor(
            f"x{g + 1}",
            [128, (bounds[g + 1] - bounds[g]) * 128],
            I8, kind="ExternalInput").ap()
        for g in range(len(bounds) - 1)
    ]

    with tile.TileContext(nc) as tc, ExitStack() as ctx:
        xpool = ctx.enter_context(tc.tile_pool(name="xpool", bufs=1))
        opool = ctx.enter_context(tc.tile_pool(name="opool", bufs=1))
        ppool = ctx.enter_context(tc.tile_pool(name="ppool", bufs=1, space="PSUM"))

        # Header (stage-1/2 weights) and scale vector ride ahead of the
        # int8 stream: header first on the sync ring, scales on the scalar
        # ring (otherwise idle).
        hdr_sb = xpool.tile([128, W4 + WNC], F16, tag="hdr")
        nc.sync.dma_start(hdr_sb[:], hdr_t[:])
        sv_sb = opool.tile([128, 1], F32, tag="sv")
        nc.scalar.dma_start(sv_sb[:], sv_t[:])

        # PE warm-up: emitted before the stream doorbells so the scheduler
        # places the gpsimd memset at the head of that engine's queue; the
        # chain then issues right after the engine prologues, several us
        # before the first x group is ready.  The zero products stay in a
        # scratch psum that is never read.
        warm = opool.tile([128, 256], F16, tag="warm")
        nc.gpsimd.memset(warm[:], 0)
        psum_w = ppool.tile([8, 256], F32, tag="warmp")
        for i in range(N_WARM):
            nc.tensor.matmul(psum_w[:], lhsT=warm[:, 0:8], rhs=warm[:],
                             start=(i == 0), stop=(i == N_WARM - 1))

        n_groups = len(bounds) - 1
        xfs = []
        for g in range(n_groups):
            ncols = (bounds[g + 1] - bounds[g]) * 128
            # Casting DMA (software DGE): the DMA path upconverts int8 ->
            # fp16 in flight, so HBM sees 1 byte/element and no compute
            # engine touches the data before the PE.
            xf = xpool.tile([128, ncols], F16, tag=f"xf{g}")
            nc.gpsimd.dma_start(xf[:], xg[g][:])
            xfs.append(xf)
        w_sb = hdr_sb[:, 0:W4]
        wn_sb = hdr_sb[0:8, W4:W4 + WNC]

        # Stage 1: 64 paired 256-column fp16 matmuls (two in flight on the
        # PE hide the ~165 ns per-instruction drain latency).  Even chunks
        # accumulate their partial S into psum quadrant (0:4, 0:128), odd
        # chunks into (4:8, 128:256); off-diagonal quadrants are garbage,
        # neutralized in stage 2 by zero rows in wn.
        psum1 = ppool.tile([8, 256], F32, tag="t")
        for g in range(n_groups):
            c0, c1 = bounds[g], bounds[g + 1]
            xf = xfs[g]
            for pp in range((c1 - c0) // 2):
                p = c0 // 2 + pp
                nc.tensor.matmul(
                    psum1[:],
                    lhsT=w_sb[:, p * 8:(p + 1) * 8],
                    rhs=xf[:, pp * 256:(pp + 1) * 256],
                    start=(p == 0),
                    stop=(p == NPAIR - 1),
                )

        # Downcast the full psum (live + garbage quadrants) to fp16 in one
        # partition-0-aligned copy; compute engines cannot start an access
        # at partition 4, so the garbage quadrants are neutralized in stage
        # 2 by zero rows in the wn operand.
        s8 = opool.tile([8, 256], F16, tag="s8")
        nc.vector.tensor_copy(s8[:], psum1[:])

        # Stage 2: contract over (k2, parity) against wn/32.
        psum2 = ppool.tile([128, 128], F32, tag="out")
        nc.tensor.matmul(psum2[:], lhsT=s8[:, 0:128], rhs=wn_sb[:, 0:128],
                         start=True, stop=False)
        nc.tensor.matmul(psum2[:], lhsT=s8[:, 128:256], rhs=wn_sb[:, 128:256],
                         start=False, stop=True)
        # Apply the per-(b,r) dequant scale on the way out: psum2 rows are
        # (b,r), so a per-partition scale vector does it in one copy.
        out_sb = opool.tile([128, 128], F32, tag="y")
        nc.scalar.mul(out_sb[:], psum2[:], sv_sb[:])
        nc.sync.dma_start(y[:], out_sb[:])

    nc.compile()
    return nc


def _prep_x(current_pose: np.ndarray):
    """(256, 4096, 16) -> per-core int8 chunk images + fp32 column scales.

    Per core the stage-1 contraction matrix has row index (j*4 + k) and
    column (b*4 + r) with element pose[b, j, 4r+k].  Chunk Jc's 128x128
    tile lands in packed columns [Jc*128, (Jc+1)*128).
    """
    a = current_pose.reshape(N_CORES, B_SH, N_IN, MPD, MPD)   # m b j r k
    t = a.transpose(0, 2, 4, 1, 3)                            # m j k b r
    X = t.reshape(N_CORES, JK, 128)                           # m (jk) (b,r)
    s = (np.abs(X).max(axis=1) / np.float32(127.0)).astype(np.float32)
    q = np.clip(np.rint(X / s[:, None, :]), -127, 127).astype(np.int8)
    c = q.reshape(N_CORES, NCHUNK, 128, 128)                  # m Jc p col
    xs = np.ascontiguousarray(
        c.transpose(0, 2, 1, 3).reshape(N_CORES, 128, XCOLS))
    return xs, s


def kernel(current_pose, w_current, w_next, h_out=1, w_out=1):
    global LAST_RESULT
    current_pose = np.asarray(current_pose, dtype=np.float32)
    w_current = np.asarray(w_current, dtype=np.float32)
    w_next = np.asarray(w_next, dtype=np.float32)

    if not TRACE:
        # bass_utils would honor a stray BASS_TRACE env var and then crash on
        # this image's missing NTFF hook module.
        os.environ.pop("BASS_TRACE", None)

    if "nc" not in _CACHE:
        _CACHE["nc"] = _build_program()
    nc = _CACHE["nc"]
    bounds = BOUNDS

    xs, s = _prep_x(current_pose)

    # wc[j,k,c] flattened over rows (j,k); chunk Jc's (128, 4) block packed
    # into header columns [Jc*4, (Jc+1)*4).
    wc_flat = w_current.reshape(JK, MPD).astype(np.float16)
    w_img = np.ascontiguousarray(
        wc_flat.reshape(NCHUNK, 128, MPD).transpose(1, 0, 2).reshape(128, W4))

    # wn arranged (k2, (o,c)), pre-scaled by the exact 1/32 softmax
    # constant, in two parity blocks: even block rows 0:4, odd block rows
    # 4:8; the complementary rows stay zero to kill the psum1 garbage
    # quadrants in stage 2.
    wn4 = (w_next.transpose(1, 0, 2).reshape(MPD, N_OUT * MPD)
           * np.float32(1.0 / N_OUT)).astype(np.float16)
    wn_img = np.zeros((128, WNC), dtype=np.float16)
    wn_img[0:MPD, 0:128] = wn4
    wn_img[MPD:2 * MPD, 128:256] = wn4
    hdr = np.ascontiguousarray(np.concatenate([w_img, wn_img], axis=1))

    in_maps = [
        {"hdr": hdr,
         "sv": np.ascontiguousarray(s[m][:, None]),
         **{f"x{g + 1}": np.ascontiguousarray(
                xs[m][:, bounds[g] * 128:bounds[g + 1] * 128])
            for g in range(len(bounds) - 1)}}
        for m in range(N_CORES)
    ]
    res = run_bass_kernel_spmd(nc, in_maps, list(range(N_CORES)), trace=TRACE,
                               **TRACE_KWARGS)
    LAST_RESULT = res

    out = np.empty((B, 1, 1, N_OUT, POSE_DIM), dtype=np.float32)
    for m in range(N_CORES):
        ym = res.results[m]["y"]                      # (128=(b,r), 128=(o,c))
        out[m * B_SH:(m + 1) * B_SH, 0, 0] = (
            ym.reshape(B_SH, MPD, N_OUT, MPD)
            .transpose(0, 2, 1, 3).reshape(B_SH, N_OUT, POSE_DIM))
    return out
